# revision 4
# baseline (speedup 1.0000x reference)
"""Trainium2 Bass kernel for masked multi-head attention + depthwise residual conv.

Reference computation (per batch b):
    qkv = x @ W_qkv -> split (3, heads=8, d=64)
    dots = q @ k^T * d**-0.5 ; key-masked softmax
    out  = attn @ v + depthwise_conv33_seq(v)
    out  = out @ W_out + b_out ; row-masked to 0

Sharding: 16 (batch, head-pair) units -> 8 cores, each core handles one batch
and two adjacent heads, producing a partial [2048, 512] projection output.
Host sums the 4 partials per batch, adds b_out, applies the row mask.

Device-side layout trick: dots are computed transposed, dotsT[j, i] (keys on
partitions), so the key mask is a per-partition bias on the exp activation and
softmax needs no max-pass (dots ~ N(0,1)); the softmax denominator l_i comes
for free from a ones-column appended to v in the attn@v matmul. The depthwise
conv is a banded-Toeplitz matmul with host-precomputed [128, 512] blocks.
"""

import sys

sys.path.insert(0, "/opt/trn_rl_repo")

from contextlib import ExitStack

import numpy as np

import concourse.bass as bass
import concourse.tile as tile
from concourse import bacc, mybir

F32 = mybir.dt.float32

HEADS = 8
D = 64
DIM = 512
KER = 33
PAD = KER // 2
SCALE = D ** -0.5
B = 2
N = 2048
NCORES = 8
NEG = -1.0e30


def _build_body(ctx: ExitStack, tc: "tile.TileContext", ins, outs):
    nc = tc.nc
    xT, wqkv, wout, tblk, mbias = ins
    out = outs[0]

    Exp = mybir.ActivationFunctionType.Exp

    singles = ctx.enter_context(tc.tile_pool(name="singles", bufs=1))
    lpool = ctx.enter_context(tc.tile_pool(name="lpool", bufs=4))
    ptpool = ctx.enter_context(tc.tile_pool(name="ptpool", bufs=3))
    opool = ctx.enter_context(tc.tile_pool(name="opool", bufs=2))
    psA = ctx.enter_context(tc.tile_pool(name="psA", bufs=2, space="PSUM"))
    psAV = ctx.enter_context(tc.tile_pool(name="psAV", bufs=2, space="PSUM"))
    psR = ctx.enter_context(tc.tile_pool(name="psR", bufs=2, space="PSUM"))

    # ---- resident SBUF tensors ----
    xt_sb = singles.tile([128, 4, N], F32)  # xT chunks: [p, kc, i]
    nc.sync.dma_start(out=xt_sb[:], in_=xT.rearrange("(c p) i -> p c i", p=128))
    w_sb = singles.tile([128, 4, 384], F32)  # W_qkv slice: [p, kc, f]
    nc.sync.dma_start(out=w_sb[:], in_=wqkv.rearrange("(c p) f -> p c f", p=128))
    wout_sb = []
    for h in range(2):
        t = singles.tile([64, DIM], F32, tag=f"wout{h}", name=f"wout_sb{h}")
        nc.sync.dma_start(out=t[:], in_=wout[h * 64 : (h + 1) * 64, :])
        wout_sb.append(t)
    tb_sb = singles.tile([128, 12, DIM], F32)  # conv blocks: [p, h*6+m, fi]
    nc.sync.dma_start(out=tb_sb[:], in_=tblk.rearrange("g p f -> p g f"))
    mb_sb = singles.tile([128, 16], F32)  # mask bias per j: [p, jc]
    nc.sync.dma_start(out=mb_sb[:], in_=mbias[:, :])

    ones_sb = singles.tile([65, 64], F32)
    nc.vector.memset(ones_sb[:], 1.0)
    v_sb = singles.tile([128, 16, 130], F32)  # [j_p, jc, 65*h + (d|1)]
    nc.vector.memset(v_sb[:], 1.0)

    qt_sb = singles.tile([128, N], F32)  # qT: rows h*64+d, cols i
    kt_sb = singles.tile([128, N], F32)
    projin = []
    for h in range(2):
        projin.append(singles.tile([64, N], F32, tag=f"projin{h}", name=f"projin{h}"))

    # ---- qT / kT: (W_{q,k})^T @ x^T, f on partitions ----
    for fc, dst in ((0, qt_sb), (1, kt_sb)):
        for ic in range(4):
            ps = psA.tile([128, 1024], F32, tag="A")
            for kc in range(4):
                nc.tensor.matmul(
                    ps[:, 0:512],
                    w_sb[:, kc, fc * 128 : (fc + 1) * 128],
                    xt_sb[:, kc, ic * 512 : (ic + 1) * 512],
                    start=(kc == 0),
                    stop=(kc == 3),
                )
            nc.vector.tensor_copy(dst[:, ic * 512 : (ic + 1) * 512], ps[:, 0:512])

    # ---- v in natural [i, d] layout (x^T chunks stationary) ----
    for jc in range(16):
        ps = psR.tile([128, 128], F32, tag="R")
        for kc in range(4):
            nc.tensor.matmul(
                ps[:],
                xt_sb[:, kc, jc * 128 : (jc + 1) * 128],
                w_sb[:, kc, 256:384],
                start=(kc == 0),
                stop=(kc == 3),
            )
        nc.vector.tensor_copy(v_sb[:, jc, 0:64], ps[:, 0:64])
        nc.vector.tensor_copy(v_sb[:, jc, 65:129], ps[:, 64:128])

    # ---- attention + conv + projection, per 512-wide i chunk ----
    for ic in range(4):
        i5 = slice(ic * 512, (ic + 1) * 512)

        # residual conv: res_h^T[d, i] = sum_j v_h[j, d] * T^T[j, i] (banded)
        resh = []
        for h in range(2):
            rps = psR.tile([64, 512], F32, tag="R")
            ms = [m for m in range(6) if 0 <= ic * 512 - 128 + m * 128 < N]
            for mi, m in enumerate(ms):
                jc = (ic * 512 - 128 + m * 128) // 128
                nc.tensor.matmul(
                    rps[:],
                    v_sb[:, jc, h * 65 : h * 65 + 64],
                    tb_sb[:, h * 6 + m, :],
                    start=(mi == 0),
                    stop=(mi == len(ms) - 1),
                )
            resh.append(rps)

        avh = []
        for h in range(2):
            avh.append(psAV.tile([65, 512], F32, tag="AV", name="av"))

        for jc in range(16):
            dots = psA.tile([128, 1024], F32, tag="A")
            for h in range(2):
                nc.tensor.matmul(
                    dots[:, h * 512 : (h + 1) * 512],
                    kt_sb[h * 64 : (h + 1) * 64, jc * 128 : (jc + 1) * 128],
                    qt_sb[h * 64 : (h + 1) * 64, i5],
                    start=True,
                    stop=True,
                )
            pt = ptpool.tile([128, 1024], F32, tag="pt")
            nc.scalar.activation(
                pt[:], dots[:], Exp, bias=mb_sb[:, jc : jc + 1], scale=SCALE
            )
            for h in range(2):
                nc.tensor.matmul(
                    avh[h][:],
                    v_sb[:, jc, h * 65 : (h + 1) * 65],
                    pt[:, h * 512 : (h + 1) * 512],
                    start=(jc == 0),
                    stop=(jc == 15),
                )

        # epilogue: normalize by l (row 64 of avh), add conv residual
        for h in range(2):
            lt = lpool.tile([65, 512], F32, tag="l")
            nc.scalar.copy(lt[64:65, :], avh[h][64:65, :])
            bc = psA.tile([64, 512], F32, tag="A")
            nc.tensor.matmul(
                bc[:], ones_sb[64:65, 0:64], lt[64:65, :], start=True, stop=True
            )
            rc = lpool.tile([64, 512], F32, tag="rc")
            nc.vector.reciprocal_approx_fast(rc[:], bc[:])
            nc.vector.tensor_mul(projin[h][:, i5], avh[h][0:64, :], rc[:])
            nc.vector.tensor_add(projin[h][:, i5], projin[h][:, i5], resh[h][:])

        # projection: out[i, :] = sum_h projin_h^T[:, i]^T @ W_out_h
        osb = opool.tile([128, 4, DIM], F32, tag="osb")
        for sub in range(4):
            pp = psA.tile([128, 512], F32, tag="A")
            i0 = ic * 512 + sub * 128
            for h in range(2):
                nc.tensor.matmul(
                    pp[:],
                    projin[h][:, i0 : i0 + 128],
                    wout_sb[h][:],
                    start=(h == 0),
                    stop=(h == 1),
                )
            nc.vector.tensor_copy(osb[:, sub, :], pp[:])
        nc.sync.dma_start(
            out=out.rearrange("(ic c p) f -> ic p c f", c=4, p=128)[ic],
            in_=osb[:],
        )


_NC_CACHE = {}


def _get_nc(reps: int = 1):
    if reps in _NC_CACHE:
        return _NC_CACHE[reps]
    nc = bacc.Bacc(
        "TRN2",
        target_bir_lowering=False,
        debug=False,
        num_devices=NCORES,
    )
    ins = [
        nc.dram_tensor("xT", [DIM, N], F32, kind="ExternalInput").ap(),
        nc.dram_tensor("wqkv", [DIM, 384], F32, kind="ExternalInput").ap(),
        nc.dram_tensor("wout", [128, DIM], F32, kind="ExternalInput").ap(),
        nc.dram_tensor("tblk", [12, 128, DIM], F32, kind="ExternalInput").ap(),
        nc.dram_tensor("mbias", [128, 16], F32, kind="ExternalInput").ap(),
    ]
    outs = [nc.dram_tensor("out", [N, DIM], F32, kind="ExternalOutput").ap()]
    with tile.TileContext(nc) as tc:
        for _ in range(reps):
            with ExitStack() as ctx:
                _build_body(ctx, tc, ins, outs)
    nc.compile()
    _NC_CACHE[reps] = nc
    return nc


def _conv_blocks(conv_w_pair: np.ndarray) -> np.ndarray:
    """[2, 33] taps -> [12, 128, 512] banded T^T blocks.

    Block (h, m) holds T^T[j0+pj, i0+fi] = w_h[(j0-i0) + pj - fi + PAD] with
    j0-i0 = -128 + 128*m, zero outside the +-PAD band.
    """
    blocks = np.zeros((2, 6, 128, DIM), np.float32)
    pj = np.arange(128)[:, None]
    fi = np.arange(DIM)[None, :]
    for h in range(2):
        w = conv_w_pair[h]
        for m in range(6):
            idx = (-128 + 128 * m) + pj - fi + PAD
            valid = (idx >= 0) & (idx < KER)
            blocks[h, m][valid] = w[idx[valid]]
    return blocks.reshape(12, 128, DIM)


def _make_in_maps(x, mask, W_qkv, W_out, conv_w):
    x = np.asarray(x, np.float32)
    mask = np.asarray(mask)
    W_qkv = np.asarray(W_qkv, np.float32)
    W_out = np.asarray(W_out, np.float32)
    conv_w = np.asarray(conv_w, np.float32)
    in_maps = []
    for core in range(NCORES):
        b = core // 4
        h0 = (core % 4) * 2
        cols = slice(h0 * 64, h0 * 64 + 128)
        wq = W_qkv[:, 0 * DIM + h0 * 64 : 0 * DIM + h0 * 64 + 128]
        wk = W_qkv[:, 1 * DIM + h0 * 64 : 1 * DIM + h0 * 64 + 128]
        wv = W_qkv[:, 2 * DIM + h0 * 64 : 2 * DIM + h0 * 64 + 128]
        mb = np.where(mask[b], 0.0, NEG).astype(np.float32)
        in_maps.append(
            {
                "xT": np.ascontiguousarray(x[b].T),
                "wqkv": np.ascontiguousarray(np.concatenate([wq, wk, wv], axis=1)),
                "wout": np.ascontiguousarray(W_out[cols, :]),
                "tblk": _conv_blocks(conv_w[h0 : h0 + 2, 0, :, 0]),
                "mbias": np.ascontiguousarray(mb.reshape(16, 128).T),
            }
        )

    return in_maps


def _combine(results, mask, b_out):
    out = np.zeros((B, N, DIM), np.float32)
    for core in range(NCORES):
        out[core // 4] += results[core]["out"]
    out += np.asarray(b_out, np.float32)[None, None, :]
    out *= np.asarray(mask)[:, :, None].astype(np.float32)
    return out


def kernel(x, mask, W_qkv, W_out, b_out, conv_w):
    from concourse.bass_utils import run_bass_kernel_spmd

    nc = _get_nc()
    in_maps = _make_in_maps(x, mask, W_qkv, W_out, conv_w)
    results = run_bass_kernel_spmd(nc, in_maps, list(range(NCORES))).results
    return _combine(results, mask, b_out)


# revision 7
# speedup vs baseline: 1.0295x; 1.0295x over previous
"""Trainium2 Bass kernel for masked multi-head attention + depthwise residual conv.

Reference computation (per batch b):
    qkv = x @ W_qkv -> split (3, heads=8, d=64)
    dots = q @ k^T * d**-0.5 ; key-masked softmax
    out  = attn @ v + depthwise_conv33_seq(v)
    out  = out @ W_out + b_out ; row-masked to 0

Sharding: 16 (batch, head-pair) units -> 8 cores, each core handles one batch
and two adjacent heads, producing a partial [2048, 512] projection output.
Host sums the 4 partials per batch, adds b_out, applies the row mask.

Device-side layout trick: dots are computed transposed, dotsT[j, i] (keys on
partitions), so the key mask is a per-partition bias on the exp activation and
softmax needs no max-pass (dots ~ N(0,1)); the softmax denominator l_i comes
for free from a ones-column appended to v in the attn@v matmul. The depthwise
conv is a banded-Toeplitz matmul with host-precomputed [128, 512] blocks.
"""

import sys

sys.path.insert(0, "/opt/trn_rl_repo")

from contextlib import ExitStack

import numpy as np

import concourse.bass as bass
import concourse.tile as tile
from concourse import bacc, mybir

F32 = mybir.dt.float32
F32R = mybir.dt.float32r

HEADS = 8
D = 64
DIM = 512
KER = 33
PAD = KER // 2
SCALE = D ** -0.5
B = 2
N = 2048
NCORES = 8
NEG = -1.0e30


def _build_body(ctx: ExitStack, tc: "tile.TileContext", ins, outs):
    nc = tc.nc
    xT, wqkv, wout, tblk, mbias, onesd = ins
    out = outs[0]

    Exp = mybir.ActivationFunctionType.Exp

    singles = ctx.enter_context(tc.tile_pool(name="singles", bufs=1))
    lpool = ctx.enter_context(tc.tile_pool(name="lpool", bufs=4))
    ptpool = ctx.enter_context(tc.tile_pool(name="ptpool", bufs=3))
    opool = ctx.enter_context(tc.tile_pool(name="opool", bufs=2))
    psA = ctx.enter_context(tc.tile_pool(name="psA", bufs=2, space="PSUM"))
    psAV = ctx.enter_context(tc.tile_pool(name="psAV", bufs=2, space="PSUM"))
    psR = ctx.enter_context(tc.tile_pool(name="psR", bufs=2, space="PSUM"))

    # ---- resident SBUF tensors ----
    xt_sb = singles.tile([128, 4, N], F32R)  # xT chunks: [p, kc, i]
    nc.sync.dma_start(out=xt_sb[:], in_=xT.rearrange("(c p) i -> p c i", p=128))
    w_sb = singles.tile([128, 4, 384], F32R)  # W_qkv slice: [p, kc, f]
    nc.sync.dma_start(out=w_sb[:], in_=wqkv.rearrange("(c p) f -> p c f", p=128))
    wout_sb = []
    for h in range(2):
        t = singles.tile([64, DIM], F32R, tag=f"wout{h}", name=f"wout_sb{h}")
        nc.sync.dma_start(out=t[:], in_=wout[h * 64 : (h + 1) * 64, :])
        wout_sb.append(t)
    tb_sb = singles.tile([128, 12, DIM], F32R)  # conv blocks: [p, h*6+m, fi]
    nc.sync.dma_start(out=tb_sb[:], in_=tblk.rearrange("g p f -> p g f"))
    mb_sb = singles.tile([128, 16], F32)  # mask bias per j: [p, jc]
    nc.sync.dma_start(out=mb_sb[:], in_=mbias[:, :])

    ones_sb = singles.tile([65, 64], F32R)
    nc.sync.dma_start(out=ones_sb[:], in_=onesd[0:65, 0:64])
    ident = singles.tile([128, 128], F32)
    from concourse.masks import make_identity

    make_identity(nc, ident[:])
    v_sb = singles.tile([128, 16, 130], F32R)  # [j_p, jc, 65*h + (d|1)]
    for jc in range(16):
        nc.sync.dma_start(out=v_sb[:, jc, :], in_=onesd[:, :])

    qt_sb = singles.tile([128, N], F32R)  # qT: rows h*64+d, cols i
    kt_sb = singles.tile([128, N], F32R)
    vt_sb = singles.tile([128, N], F32)
    projin = []
    for h in range(2):
        projin.append(singles.tile([64, N], F32R, tag=f"projin{h}", name=f"projin{h}"))

    # ---- qT / kT / vT: W^T @ x^T, f on partitions ----
    for fc, dst in ((0, qt_sb), (1, kt_sb), (2, vt_sb)):
        for ic in range(4):
            ps = psA.tile([128, 1024], F32, tag="A")
            for kc in range(4):
                nc.tensor.matmul(
                    ps[:, 0:512],
                    w_sb[:, kc, fc * 128 : (fc + 1) * 128],
                    xt_sb[:, kc, ic * 512 : (ic + 1) * 512],
                    start=(kc == 0),
                    stop=(kc == 3),
                )
            nc.vector.tensor_copy(dst[:, ic * 512 : (ic + 1) * 512], ps[:, 0:512])

    # ---- v natural [i, d]: PE-transpose of vT 128x128 blocks ----
    for jc in range(16):
        ps = psR.tile([128, 128], F32, tag="R")
        nc.tensor.transpose(ps[:], vt_sb[:, jc * 128 : (jc + 1) * 128], ident[:])
        nc.vector.tensor_copy(v_sb[:, jc, 0:64], ps[:, 0:64])
        nc.vector.tensor_copy(v_sb[:, jc, 65:129], ps[:, 64:128])

    # ---- attention + conv + projection, per 512-wide i chunk ----
    for ic in range(4):
        i5 = slice(ic * 512, (ic + 1) * 512)

        # residual conv: res_h^T[d, i] = sum_j v_h[j, d] * T^T[j, i] (banded)
        resh = []
        for h in range(2):
            rps = psR.tile([64, 512], F32, tag="R")
            ms = [m for m in range(6) if 0 <= ic * 512 - 128 + m * 128 < N]
            for mi, m in enumerate(ms):
                jc = (ic * 512 - 128 + m * 128) // 128
                nc.tensor.matmul(
                    rps[:],
                    v_sb[:, jc, h * 65 : h * 65 + 64],
                    tb_sb[:, h * 6 + m, :],
                    start=(mi == 0),
                    stop=(mi == len(ms) - 1),
                )
            resh.append(rps)

        avh = []
        for h in range(2):
            avh.append(psAV.tile([65, 512], F32, tag="AV", name="av"))

        for jc in range(16):
            dots = psA.tile([128, 1024], F32, tag="A")
            for h in range(2):
                nc.tensor.matmul(
                    dots[:, h * 512 : (h + 1) * 512],
                    kt_sb[h * 64 : (h + 1) * 64, jc * 128 : (jc + 1) * 128],
                    qt_sb[h * 64 : (h + 1) * 64, i5],
                    start=True,
                    stop=True,
                )
            pt = ptpool.tile([128, 1024], F32R, tag="pt")
            nc.scalar.activation(
                pt[:], dots[:], Exp, bias=mb_sb[:, jc : jc + 1], scale=SCALE
            )
            for h in range(2):
                nc.tensor.matmul(
                    avh[h][:],
                    v_sb[:, jc, h * 65 : (h + 1) * 65],
                    pt[:, h * 512 : (h + 1) * 512],
                    start=(jc == 0),
                    stop=(jc == 15),
                )

        # epilogue: normalize by l (row 64 of avh), add conv residual
        for h in range(2):
            lt = lpool.tile([65, 512], F32R, tag="l")
            nc.scalar.copy(lt[64:65, :], avh[h][64:65, :])
            bc = psA.tile([64, 512], F32, tag="A")
            nc.tensor.matmul(
                bc[:], ones_sb[64:65, 0:64], lt[64:65, :], start=True, stop=True
            )
            rc = lpool.tile([64, 512], F32, tag="rc")
            nc.vector.reciprocal_approx_fast(rc[:], bc[:])
            nc.vector.tensor_mul(projin[h][:, i5], avh[h][0:64, :], rc[:])
            nc.vector.tensor_add(projin[h][:, i5], projin[h][:, i5], resh[h][:])

        # projection: out[i, :] = sum_h projin_h^T[:, i]^T @ W_out_h
        osb = opool.tile([128, 4, DIM], F32, tag="osb")
        for sub in range(4):
            pp = psA.tile([128, 512], F32, tag="A")
            i0 = ic * 512 + sub * 128
            for h in range(2):
                nc.tensor.matmul(
                    pp[:],
                    projin[h][:, i0 : i0 + 128],
                    wout_sb[h][:],
                    start=(h == 0),
                    stop=(h == 1),
                )
            nc.vector.tensor_copy(osb[:, sub, :], pp[:])
        nc.sync.dma_start(
            out=out.rearrange("(ic c p) f -> ic p c f", c=4, p=128)[ic],
            in_=osb[:],
        )


_NC_CACHE = {}


def _get_nc(reps: int = 1):
    if reps in _NC_CACHE:
        return _NC_CACHE[reps]
    nc = bacc.Bacc(
        "TRN2",
        target_bir_lowering=False,
        debug=False,
        num_devices=NCORES,
    )
    ins = [
        nc.dram_tensor("xT", [DIM, N], F32R, kind="ExternalInput").ap(),
        nc.dram_tensor("wqkv", [DIM, 384], F32R, kind="ExternalInput").ap(),
        nc.dram_tensor("wout", [128, DIM], F32R, kind="ExternalInput").ap(),
        nc.dram_tensor("tblk", [12, 128, DIM], F32R, kind="ExternalInput").ap(),
        nc.dram_tensor("mbias", [128, 16], F32, kind="ExternalInput").ap(),
        nc.dram_tensor("ones", [128, 130], F32R, kind="ExternalInput").ap(),
    ]
    outs = [nc.dram_tensor("out", [N, DIM], F32, kind="ExternalOutput").ap()]
    with tile.TileContext(nc) as tc:
        for _ in range(reps):
            with ExitStack() as ctx:
                _build_body(ctx, tc, ins, outs)
    nc.compile()
    _NC_CACHE[reps] = nc
    return nc


def _conv_blocks(conv_w_pair: np.ndarray) -> np.ndarray:
    """[2, 33] taps -> [12, 128, 512] banded T^T blocks.

    Block (h, m) holds T^T[j0+pj, i0+fi] = w_h[(j0-i0) + pj - fi + PAD] with
    j0-i0 = -128 + 128*m, zero outside the +-PAD band.
    """
    blocks = np.zeros((2, 6, 128, DIM), np.float32)
    pj = np.arange(128)[:, None]
    fi = np.arange(DIM)[None, :]
    for h in range(2):
        w = conv_w_pair[h]
        for m in range(6):
            idx = (-128 + 128 * m) + pj - fi + PAD
            valid = (idx >= 0) & (idx < KER)
            blocks[h, m][valid] = w[idx[valid]]
    return blocks.reshape(12, 128, DIM)


def _make_in_maps(x, mask, W_qkv, W_out, conv_w):
    x = np.asarray(x, np.float32)
    mask = np.asarray(mask)
    W_qkv = np.asarray(W_qkv, np.float32)
    W_out = np.asarray(W_out, np.float32)
    conv_w = np.asarray(conv_w, np.float32)
    in_maps = []
    for core in range(NCORES):
        b = core // 4
        h0 = (core % 4) * 2
        cols = slice(h0 * 64, h0 * 64 + 128)
        wq = W_qkv[:, 0 * DIM + h0 * 64 : 0 * DIM + h0 * 64 + 128]
        wk = W_qkv[:, 1 * DIM + h0 * 64 : 1 * DIM + h0 * 64 + 128]
        wv = W_qkv[:, 2 * DIM + h0 * 64 : 2 * DIM + h0 * 64 + 128]
        mb = np.where(mask[b], 0.0, NEG).astype(np.float32)
        in_maps.append(
            {
                "xT": np.ascontiguousarray(x[b].T),
                "wqkv": np.ascontiguousarray(np.concatenate([wq, wk, wv], axis=1)),
                "wout": np.ascontiguousarray(W_out[cols, :]),
                "tblk": _conv_blocks(conv_w[h0 : h0 + 2, 0, :, 0]),
                "mbias": np.ascontiguousarray(mb.reshape(16, 128).T),
                "ones": np.ones((128, 130), np.float32),
            }
        )

    return in_maps


def _combine(results, mask, b_out):
    out = np.zeros((B, N, DIM), np.float32)
    for core in range(NCORES):
        out[core // 4] += results[core]["out"]
    out += np.asarray(b_out, np.float32)[None, None, :]
    out *= np.asarray(mask)[:, :, None].astype(np.float32)
    return out


def kernel(x, mask, W_qkv, W_out, b_out, conv_w):
    from concourse.bass_utils import run_bass_kernel_spmd

    nc = _get_nc()
    in_maps = _make_in_maps(x, mask, W_qkv, W_out, conv_w)
    results = run_bass_kernel_spmd(nc, in_maps, list(range(NCORES))).results
    return _combine(results, mask, b_out)


# revision 11
# speedup vs baseline: 15.0949x; 14.6630x over previous
"""Trainium2 Bass kernel for masked multi-head attention + depthwise residual conv.

Reference computation (per batch b):
    qkv = x @ W_qkv -> split (3, heads=8, d=64)
    dots = q @ k^T * d**-0.5 ; key-masked softmax
    out  = attn @ v + depthwise_conv33_seq(v)
    out  = out @ W_out + b_out ; row-masked to 0

Sharding: 16 (batch, head-pair) units -> 8 cores, each core handles one batch
and two adjacent heads, producing a partial [2048, 512] projection output.
Host sums the 4 partials per batch, adds b_out, applies the row mask.

Device-side layout trick: dots are computed transposed, dotsT[j, i] (keys on
partitions), so the key mask is a per-partition bias on the exp activation and
softmax needs no max-pass (dots ~ N(0,1)); the softmax denominator l_i comes
for free from a ones-column appended to v in the attn@v matmul. The depthwise
conv is a banded-Toeplitz matmul with host-precomputed [128, 512] blocks.
"""

import sys

sys.path.insert(0, "/opt/trn_rl_repo")

from contextlib import ExitStack

import numpy as np

import concourse.bass as bass
import concourse.tile as tile
from concourse import bacc, mybir

F32 = mybir.dt.float32
F32R = mybir.dt.float32r

HEADS = 8
D = 64
DIM = 512
KER = 33
PAD = KER // 2
SCALE = D ** -0.5
B = 2
N = 2048
NCORES = 8
NEG = -1.0e30


def _build_body(ctx: ExitStack, tc: "tile.TileContext", ins, outs):
    nc = tc.nc
    xT, wqkv, wout, tblk, mbias, onesd, onesd2080 = ins
    out = outs[0]

    Exp = mybir.ActivationFunctionType.Exp

    singles = ctx.enter_context(tc.tile_pool(name="singles", bufs=1))
    lpool = ctx.enter_context(tc.tile_pool(name="lpool", bufs=4))
    ptpool = ctx.enter_context(tc.tile_pool(name="ptpool", bufs=3))
    opool = ctx.enter_context(tc.tile_pool(name="opool", bufs=2))
    psA = ctx.enter_context(tc.tile_pool(name="psA", bufs=2, space="PSUM"))
    psAV = ctx.enter_context(tc.tile_pool(name="psAV", bufs=2, space="PSUM"))
    psR = ctx.enter_context(tc.tile_pool(name="psR", bufs=2, space="PSUM"))

    # ---- resident SBUF tensors ----
    xt_sb = singles.tile([128, 4, N], F32R)  # xT chunks: [p, kc, i]
    xTr = xT.rearrange("(c p) i -> c p i", p=128)
    for kc in range(4):
        nc.sync.dma_start(out=xt_sb[:, kc, :], in_=xTr[kc])
    w_sb = singles.tile([128, 4, 384], F32R)  # W_qkv slice: [p, kc, f]
    nc.sync.dma_start(out=w_sb[:], in_=wqkv.rearrange("(c p) f -> p c f", p=128))
    wout_sb = []
    for h in range(2):
        t = singles.tile([64, DIM], F32R, tag=f"wout{h}", name=f"wout_sb{h}")
        nc.sync.dma_start(out=t[:], in_=wout[h * 64 : (h + 1) * 64, :])
        wout_sb.append(t)
    tb_sb = singles.tile([128, 12, DIM], F32R)  # conv blocks: [p, h*6+m, fi]
    nc.sync.dma_start(out=tb_sb[:], in_=tblk.rearrange("g p f -> p g f"))
    mb_sb = singles.tile([128, 16], F32)  # mask bias per j: [p, jc]
    nc.sync.dma_start(out=mb_sb[:], in_=mbias[:, :])

    ones_sb = singles.tile([65, 64], F32R)
    nc.sync.dma_start(out=ones_sb[:], in_=onesd[0:65, 0:64])
    v_ones_done = True
    ident = singles.tile([128, 128], F32)
    from concourse.masks import make_identity

    make_identity(nc, ident[:])
    v_sb = singles.tile([128, 16, 130], F32R)  # [j_p, jc, 65*h + (d|1)]
    nc.sync.dma_start(out=v_sb[:, :, :], in_=onesd2080[:, :])

    qt_sb = singles.tile([128, N], F32R)  # qT: rows h*64+d, cols i
    kt_sb = singles.tile([128, N], F32R)
    vt_sb = singles.tile([128, N], F32)
    projin = []
    res_sb = []
    av_sb = []
    for h in range(2):
        projin.append(singles.tile([64, N], F32R, tag=f"projin{h}", name=f"projin{h}"))
        res_sb.append(singles.tile([64, N], F32, tag=f"res_sb{h}", name=f"res_sb{h}"))
        av_sb.append(singles.tile([65, N], F32R, tag=f"av_sb{h}", name=f"av_sb{h}"))

    # ---- qT / kT / vT: W^T @ x^T, f on partitions ----
    for fc, dst in ((0, qt_sb), (1, kt_sb), (2, vt_sb)):
        for ic in range(4):
            ps = psA.tile([128, 1024], F32, tag="A")
            for kc in range(4):
                nc.tensor.matmul(
                    ps[:, 0:512],
                    w_sb[:, kc, fc * 128 : (fc + 1) * 128],
                    xt_sb[:, kc, ic * 512 : (ic + 1) * 512],
                    start=(kc == 0),
                    stop=(kc == 3),
                )
            nc.vector.tensor_copy(dst[:, ic * 512 : (ic + 1) * 512], ps[:, 0:512])

    # ---- v natural [i, d]: PE-transpose of vT 128x128 blocks ----
    for jc in range(16):
        ps = psR.tile([128, 128], F32, tag="R")
        nc.tensor.transpose(ps[:], vt_sb[:, jc * 128 : (jc + 1) * 128], ident[:])
        nc.vector.tensor_copy(v_sb[:, jc, 0:64], ps[:, 0:64])
        nc.vector.tensor_copy(v_sb[:, jc, 65:129], ps[:, 64:128])

    # ---- attention + conv + projection, per 512-wide i chunk ----
    for ic in range(4):
        i5 = slice(ic * 512, (ic + 1) * 512)

        # residual conv: res_h^T[d, i] = sum_j v_h[j, d] * T^T[j, i] (banded)
        resh = []
        for h in range(2):
            rps = psR.tile([64, 512], F32, tag="R")
            ms = [m for m in range(6) if 0 <= ic * 512 - 128 + m * 128 < N]
            for mi, m in enumerate(ms):
                jc = (ic * 512 - 128 + m * 128) // 128
                nc.tensor.matmul(
                    rps[:],
                    v_sb[:, jc, h * 65 : h * 65 + 64],
                    tb_sb[:, h * 6 + m, :],
                    start=(mi == 0),
                    stop=(mi == len(ms) - 1),
                )
            nc.vector.tensor_copy(res_sb[h][:, i5], rps[:])
            resh.append(rps)

        avh = []
        for h in range(2):
            avh.append(psAV.tile([65, 512], F32, tag="AV", name="av"))

        for jc in range(16):
            dots = psA.tile([128, 1024], F32, tag="A")
            for h in range(2):
                nc.tensor.matmul(
                    dots[:, h * 512 : (h + 1) * 512],
                    kt_sb[h * 64 : (h + 1) * 64, jc * 128 : (jc + 1) * 128],
                    qt_sb[h * 64 : (h + 1) * 64, i5],
                    start=True,
                    stop=True,
                )
            pt = ptpool.tile([128, 1024], F32R, tag="pt")
            nc.scalar.activation(
                pt[:], dots[:], Exp, bias=mb_sb[:, jc : jc + 1], scale=SCALE
            )
            for h in range(2):
                nc.tensor.matmul(
                    avh[h][:],
                    v_sb[:, jc, h * 65 : (h + 1) * 65],
                    pt[:, h * 512 : (h + 1) * 512],
                    start=(jc == 0),
                    stop=(jc == 15),
                )

        # epilogue: normalize by l (row 64 of av), add conv residual
        for h in range(2):
            nc.vector.tensor_copy(av_sb[h][:, i5], avh[h][:])
        for h in range(2):
            bc = psA.tile([64, 512], F32, tag="A")
            nc.tensor.matmul(
                bc[:],
                ones_sb[64:65, 0:64],
                av_sb[h][64:65, i5],
                start=True,
                stop=True,
            )
            rc = lpool.tile([64, 512], F32, tag="rc")
            nc.vector.reciprocal_approx_fast(rc[:], bc[:])
            nc.vector.tensor_mul(projin[h][:, i5], av_sb[h][0:64, i5], rc[:])
            nc.vector.tensor_add(projin[h][:, i5], projin[h][:, i5], res_sb[h][:, i5])

        # projection: out[i, :] = sum_h projin_h^T[:, i]^T @ W_out_h
        osb = opool.tile([128, 4, DIM], F32, tag="osb")
        for sub in range(4):
            pp = psA.tile([128, 512], F32, tag="A")
            i0 = ic * 512 + sub * 128
            for h in range(2):
                nc.tensor.matmul(
                    pp[:],
                    projin[h][:, i0 : i0 + 128],
                    wout_sb[h][:],
                    start=(h == 0),
                    stop=(h == 1),
                )
            nc.vector.tensor_copy(osb[:, sub, :], pp[:])
        nc.sync.dma_start(
            out=out.rearrange("(ic c p) f -> ic p c f", c=4, p=128)[ic],
            in_=osb[:],
        )


_NC_CACHE = {}


def _get_nc(reps: int = 1):
    if reps in _NC_CACHE:
        return _NC_CACHE[reps]
    nc = bacc.Bacc(
        "TRN2",
        target_bir_lowering=False,
        debug=False,
        num_devices=NCORES,
    )
    ins = [
        nc.dram_tensor("xT", [DIM, N], F32R, kind="ExternalInput").ap(),
        nc.dram_tensor("wqkv", [DIM, 384], F32R, kind="ExternalInput").ap(),
        nc.dram_tensor("wout", [128, DIM], F32R, kind="ExternalInput").ap(),
        nc.dram_tensor("tblk", [12, 128, DIM], F32R, kind="ExternalInput").ap(),
        nc.dram_tensor("mbias", [128, 16], F32, kind="ExternalInput").ap(),
        nc.dram_tensor("ones", [128, 130], F32R, kind="ExternalInput").ap(),
        nc.dram_tensor("ones2080", [128, 2080], F32R, kind="ExternalInput").ap(),
    ]
    outs = [nc.dram_tensor("out", [N, DIM], F32, kind="ExternalOutput").ap()]
    with tile.TileContext(nc) as tc:
        if reps == 1:
            with ExitStack() as ctx:
                _build_body(ctx, tc, ins, outs)
        else:
            with tc.For_i(0, reps, 1):
                with ExitStack() as ctx:
                    _build_body(ctx, tc, ins, outs)
    nc.compile()
    _NC_CACHE[reps] = nc
    return nc


def _conv_blocks(conv_w_pair: np.ndarray) -> np.ndarray:
    """[2, 33] taps -> [12, 128, 512] banded T^T blocks.

    Block (h, m) holds T^T[j0+pj, i0+fi] = w_h[(j0-i0) + pj - fi + PAD] with
    j0-i0 = -128 + 128*m, zero outside the +-PAD band.
    """
    blocks = np.zeros((2, 6, 128, DIM), np.float32)
    pj = np.arange(128)[:, None]
    fi = np.arange(DIM)[None, :]
    for h in range(2):
        w = conv_w_pair[h]
        for m in range(6):
            idx = (-128 + 128 * m) + pj - fi + PAD
            valid = (idx >= 0) & (idx < KER)
            blocks[h, m][valid] = w[idx[valid]]
    return blocks.reshape(12, 128, DIM)


def _make_in_maps(x, mask, W_qkv, W_out, conv_w):
    x = np.asarray(x, np.float32)
    mask = np.asarray(mask)
    W_qkv = np.asarray(W_qkv, np.float32)
    W_out = np.asarray(W_out, np.float32)
    conv_w = np.asarray(conv_w, np.float32)
    in_maps = []
    for core in range(NCORES):
        b = core // 4
        h0 = (core % 4) * 2
        cols = slice(h0 * 64, h0 * 64 + 128)
        wq = W_qkv[:, 0 * DIM + h0 * 64 : 0 * DIM + h0 * 64 + 128]
        wk = W_qkv[:, 1 * DIM + h0 * 64 : 1 * DIM + h0 * 64 + 128]
        wv = W_qkv[:, 2 * DIM + h0 * 64 : 2 * DIM + h0 * 64 + 128]
        mb = np.where(mask[b], 0.0, NEG).astype(np.float32)
        in_maps.append(
            {
                "xT": np.ascontiguousarray(x[b].T),
                "wqkv": np.ascontiguousarray(np.concatenate([wq, wk, wv], axis=1)),
                "wout": np.ascontiguousarray(W_out[cols, :]),
                "tblk": _conv_blocks(conv_w[h0 : h0 + 2, 0, :, 0]),
                "mbias": np.ascontiguousarray(mb.reshape(16, 128).T),
                "ones": np.ones((128, 130), np.float32),
                "ones2080": np.ones((128, 2080), np.float32),
            }
        )

    return in_maps


def _combine(results, mask, b_out):
    out = np.zeros((B, N, DIM), np.float32)
    for core in range(NCORES):
        out[core // 4] += results[core]["out"]
    out += np.asarray(b_out, np.float32)[None, None, :]
    out *= np.asarray(mask)[:, :, None].astype(np.float32)
    return out


def kernel(x, mask, W_qkv, W_out, b_out, conv_w):
    from concourse.bass_utils import run_bass_kernel_spmd

    nc = _get_nc()
    in_maps = _make_in_maps(x, mask, W_qkv, W_out, conv_w)
    results = run_bass_kernel_spmd(nc, in_maps, list(range(NCORES))).results
    return _combine(results, mask, b_out)
